# revision 39
# baseline (speedup 1.0000x reference)
"""MnistDenseBNN Trainium2 kernel: 3-layer binarized MLP, data-parallel over 8 cores.

net: h1 = sign(BN1(x @ sign(W1).T + b1))
     h2 = sign(BN2(h1 @ sign(W2).T + b2))
     out = BN3(h2 @ sign(W3).T + b3)

Strategy (v3):
 - Shard batch (16384) 8 ways; replicate binarized weights + folded BN params.
 - PE streams 1 col/cycle regardless of dtype; fp8 DoubleRow packs K=256 per
   stream. Optimization = minimize 512-col streams per (m-tile, n-tile).
 - Layer 1 (10 streams/(m,n), was 14): main plane = rtn-to-11-mantissa-bits x
   (PE fp32r truncates to 11 explicit bits, so pre-rounded values pass exactly)
   over rows 0:784; pad rows 784:896 carry the residual of rows 672:784
   (exactly representable in 11 bits). Rows 0:672's residual goes through a
   fp8e4m3 DoubleRow correction (values r*2^9, weights +-2^-9) in 3 streams.
   Final-output rel err ~1.1e-2 vs the 2e-2 gate (verified in simulation).
 - Layer 2 (16 streams): +-1 x +-1 fp8e4m3 DoubleRow, K=256/stream. At floor.
 - Layer 3: fp8 DoubleRow too (16 streams/n-tile, was 32), fused into L2 loop.
 - BN+sign fused into one ScalarE activation (Sign(p*scale + bias)) from PSUM.
"""

import sys

sys.path.insert(0, "/opt/trn_rl_repo")

import numpy as np
import ml_dtypes
from ml_dtypes import bfloat16

import concourse.bass as bass
import concourse.bacc as bacc
import concourse.mybir as mybir
from concourse.tile import TileContext
from concourse.bass_utils import run_bass_kernel_spmd

N_CORES = 8
B, IN, H, OUT = 16384, 784, 4096, 10
BC = B // N_CORES          # 2048 batch per core
K1 = 7                     # 896 = 7*128 fp32r k-tiles for layer 1 main plane
KP = K1 * 128
NCORR = 672                # rows 0:672 corrected via fp8 DR; 672:784 exact via pad
KCS = (128, 128, 80)       # corr DR chunk partition sizes (pairs): 256+256+160 rows
M1 = H // 128              # 32 hidden tiles
KT2 = M1 // 2              # 16 DoubleRow k-steps in layer 2
OPAD = 16                  # layer-3 output slot padded 10 -> 16 (DR j-stride %16)
EPS = 1e-5

f32 = mybir.dt.float32
f32r = mybir.dt.float32r
bf16 = mybir.dt.bfloat16
fp8 = mybir.dt.float8e4
fp8e5 = mybir.dt.float8e5
AF = mybir.ActivationFunctionType
DR = mybir.MatmulPerfMode.DoubleRow

_cache = {}


def _ntiles(CH):
    # split a sub-chunk into n-tiles of 512 (tail 256)
    out, ns = [], 0
    while ns < CH:
        nw = min(512, CH - ns)
        out.append((ns, nw))
        ns += nw
    return out


NCH = BC // 512            # 4 chunks of 512 per core


def _build():
    nc = bacc.Bacc()
    # x main plane, chunk-contiguous: [p, g*K1*512 + k*512 + c] = xb[k*128+p, g*512+c]
    x0d = nc.dram_tensor("x0", [128, K1 * BC], f32r, kind="ExternalInput")
    # corr planes, chunk-contiguous DR pairs:
    # [p, g*3072 + c*1024 + j*512 + cc] = r[c*256 + j*KCS[c] + p, g*512+cc]*2^12
    xcd = nc.dram_tensor("xc", [128, 6 * BC], fp8, kind="ExternalInput")
    w1 = nc.dram_tensor("w1", [M1, 128, KP], f32r, kind="ExternalInput")
    # corr weights: [m, ki, c*256 + j*128 + mf] = sign(W1)[m*128+mf, k(c,j,ki)]*2^-12
    # e5m2 so +-2^-12 is a NORMAL fp8 value (e4m3 bottoms out at 2^-9; the PE
    # mangles subnormal fp8 operands)
    w1c = nc.dram_tensor("w1c", [M1, 128, 768], fp8e5, kind="ExternalInput")
    # w2: DoubleRow-paired fp8: [m, ki, kt*256 + j*128 + mf]
    #     = sign(W2)[m*128+mf, (2*kt+j)*128 + ki]
    w2 = nc.dram_tensor("w2", [M1, 128, H], fp8, kind="ExternalInput")
    # w3 DR-paired: [kp, q*32 + j*16 + o] = sign(W3)[o, (2q+j)*128 + kp] (o<10)
    w3 = nc.dram_tensor("w3", [128, KT2 * 2 * OPAD], fp8, kind="ExternalInput")
    # packed BN consts: cols [0:32]=s1 [32:64]=t1 [64:96]=s2 [96:128]=t2
    # col 128 rows 0:10 = s3, col 129 rows 0:10 = t3
    cst = nc.dram_tensor("cst", [128, 4 * M1 + 2], f32, kind="ExternalInput")
    out = nc.dram_tensor("out", [OUT, BC], f32, kind="ExternalOutput")

    with TileContext(nc) as tc:
        with (
            tc.tile_pool(name="consts", bufs=1) as cpool,
            tc.tile_pool(name="w1p", bufs=8) as w1pool,
            tc.tile_pool(name="w1cp", bufs=16) as w1cpool,
            tc.tile_pool(name="w2p", bufs=4) as w2pool,
            tc.tile_pool(name="xs", bufs=3) as xspool,
            tc.tile_pool(name="xc", bufs=4) as xcpool,
            tc.tile_pool(name="a1", bufs=2 * KT2) as a1pool,
            tc.tile_pool(name="a2", bufs=6) as a2pool,
            tc.tile_pool(name="ob", bufs=4) as opool,
            tc.tile_pool(name="ps1", bufs=3, space="PSUM") as ps1pool,
            tc.tile_pool(name="ps2", bufs=4, space="PSUM") as ps2pool,
            tc.tile_pool(name="ps3", bufs=1, space="PSUM") as ps3pool,
        ):
            # PE warmup on junk SBUF during the DMA head: HAM un-throttles
            # (1.2 -> 2.4 GHz) after ~3.4us of sustained matmul activity, so
            # the first real matmuls run warm instead of paying the cold ramp
            wup = cpool.tile([128, 512], bf16, tag="wup")
            nc.gpsimd.memset(wup, 0)
            psw = ps1pool.tile([128, 512], f32, tag="ps1", name="ps_warm")
            for i in range(60):
                nc.tensor.matmul(
                    psw, wup[:, :128], wup, start=(i == 0), stop=(i == 59)
                )

            # head: chunk-0 x and the first weight tiles go FIRST — the Sync
            # engine issues DMAs serially at ~0.6us each, so emission order
            # sets the critical path to the first matmul
            xst0 = xspool.tile([128, K1 * 512], f32r, tag="x0", name="xs_0")
            for h in range(K1):
                nc.sync.dma_start(
                    xst0[:, h * 512 : (h + 1) * 512], x0d[:, h * 512 : (h + 1) * 512]
                )
            w1t0 = w1pool.tile([128, KP], f32r, tag="w1")
            nc.sync.dma_start(w1t0, w1[0, :, :])
            # second HWDGE ring (Activation engine) issues in parallel
            w1ct0 = w1cpool.tile([128, 768], fp8e5, tag="w1c")
            nc.scalar.dma_start(w1ct0, w1c[0, :, :])
            xct0 = xcpool.tile([128, 6 * 512], fp8, tag="xc", name="xc_0")
            for h in range(3):
                nc.scalar.dma_start(
                    xct0[:, h * 1024 : (h + 1) * 1024],
                    xcd[:, h * 1024 : (h + 1) * 1024],
                )

            w3t = cpool.tile([128, KT2 * 2 * OPAD], fp8, tag="w3")
            nc.sync.dma_start(w3t, w3[:, :])
            cstt = cpool.tile([128, 4 * M1 + 2], f32, tag="cst")
            nc.sync.dma_start(cstt, cst[:, :])
            s1t = cstt[:, 0:M1]
            t1t = cstt[:, M1 : 2 * M1]
            s2t = cstt[:, 2 * M1 : 3 * M1]
            t2t = cstt[:, 3 * M1 : 4 * M1]
            s3t = cstt[:OUT, 4 * M1 : 4 * M1 + 1]
            t3t = cstt[:OUT, 4 * M1 + 1 : 4 * M1 + 2]
            # prime the scalar engine's clock on the const DMA queue so later
            # activations carry only their PE wait (1-wait walrus limit)
            prim = cpool.tile([128, 1], f32, tag="prim")
            nc.scalar.activation(prim, cstt[:, :1], AF.Copy)

            # 4 chunks of 512 batch-cols; each chunk's x arrives as ONE fp32r DMA
            # + ONE fp8 DMA (chunk-contiguous host layout)
            for g in range(NCH):
                cs = g * 512
                # ---- layer-1 input: fp32r main plane + fp8 DR corr planes ----
                if g == 0:
                    xst, xct = xst0, xct0
                else:
                    xst = xspool.tile(
                        [128, K1 * 512], f32r, tag="x0", name=f"xs_{g}"
                    )
                    for h in range(K1):
                        nc.sync.dma_start(
                            xst[:, h * 512 : (h + 1) * 512],
                            x0d[
                                :,
                                g * K1 * 512 + h * 512 : g * K1 * 512 + (h + 1) * 512,
                            ],
                        )
                    xct = xcpool.tile([128, 6 * 512], fp8, tag="xc", name=f"xc_{g}")
                    for h in range(3):
                        nc.scalar.dma_start(
                            xct[:, h * 1024 : (h + 1) * 1024],
                            xcd[:, g * 3072 + h * 1024 : g * 3072 + (h + 1) * 1024],
                        )

                # ---- layer 1: a1 = sign(s1*(x @ W1b^T) + t1), fp8 pair planes ----
                a1tiles = []
                for m in range(M1):
                    if g == 0 and m == 0:
                        w1t, w1ct = w1t0, w1ct0
                    else:
                        w1t = w1pool.tile([128, KP], f32r, tag="w1")
                        nc.sync.dma_start(w1t, w1[m, :, :])
                        w1ct = w1cpool.tile([128, 768], fp8e5, tag="w1c")
                        nc.scalar.dma_start(w1ct, w1c[m, :, :])
                    if m % 2 == 0:
                        a1tiles.append(
                            a1pool.tile(
                                [128, 2, 512], fp8, tag="a1",
                                name=f"a1dr_{g}_{m // 2}",
                            )
                        )
                    ps = ps1pool.tile([128, 512], f32, tag="ps1")
                    for k in range(K1):
                        nc.tensor.matmul(
                            ps,
                            w1t[:, k * 128 : (k + 1) * 128],
                            xst[:, k * 512 : (k + 1) * 512],
                            start=(k == 0),
                            stop=False,
                        )
                    for c in range(3):
                        kc = KCS[c]
                        nc.tensor.matmul(
                            ps,
                            w1ct[0:kc, c * 256 : (c + 1) * 256].rearrange(
                                "p (j m) -> p j m", j=2
                            ),
                            xct[0:kc, c * 1024 : (c + 1) * 1024].rearrange(
                                "p (j n) -> p j n", j=2
                            ),
                            start=False,
                            stop=(c == 2),
                            perf_mode=DR,
                        )
                    nc.scalar.activation(
                        a1tiles[m // 2][:, m % 2, :], ps, AF.Sign,
                        bias=t1t[:, m : m + 1], scale=s1t[:, m : m + 1],
                    )

                # ---- layer 2 (fp8 DoubleRow) + fused layer 3 (fp8 DR) ----
                psum3 = ps3pool.tile([OPAD, 512], f32, tag="ps3", name=f"ps3_{g}")
                a2cur = None
                for m in range(M1):
                    q = m // 2
                    # two half-tiles so same-tag DMA spacing stays 8-aligned
                    w2a = w2pool.tile([128, H // 2], fp8, tag="w2a")
                    nc.sync.dma_start(w2a, w2[m, :, : H // 2])
                    w2b = w2pool.tile([128, H // 2], fp8, tag="w2b")
                    nc.scalar.dma_start(w2b, w2[m, :, H // 2 :])
                    halves = (w2a, w2b)
                    ps = ps2pool.tile([128, 512], f32, tag="ps2")
                    for kt in range(KT2):
                        wt = halves[kt // (KT2 // 2)]
                        kk = kt % (KT2 // 2)
                        lhs3d = wt[:, kk * 256 : (kk + 1) * 256].rearrange(
                            "p (j m) -> p j m", j=2
                        )
                        rhs3d = a1tiles[kt][:, :, :]
                        nc.tensor.matmul(
                            ps, lhs3d, rhs3d,
                            start=(kt == 0), stop=(kt == KT2 - 1),
                            perf_mode=DR,
                        )
                    if m % 2 == 0:
                        a2cur = a2pool.tile(
                            [128, 2, 512], fp8, tag="a2", name=f"a2_{g}_{q}"
                        )
                    nc.scalar.activation(
                        a2cur[:, m % 2, :], ps, AF.Sign,
                        bias=t2t[:, m : m + 1], scale=s2t[:, m : m + 1],
                    )
                    if m % 2 == 1:
                        nc.tensor.matmul(
                            psum3,
                            w3t[:, q * 2 * OPAD : (q + 1) * 2 * OPAD].rearrange(
                                "p (j o) -> p j o", j=2
                            ),
                            a2cur[:, :, :],
                            start=(q == 0),
                            stop=(q == KT2 - 1),
                            perf_mode=DR,
                        )

                ob = opool.tile([OUT, 512], f32, tag="ob")
                nc.scalar.activation(
                    ob, psum3[:OUT, :], AF.Identity,
                    bias=t3t, scale=s3t,
                )
                nc.sync.dma_start(out[:, cs : cs + 512], ob)

    nc.finalize()
    return nc


def _fold_bn(g, be, mu, va, b):
    s = g.astype(np.float64) / np.sqrt(va.astype(np.float64) + EPS)
    t = (b.astype(np.float64) - mu.astype(np.float64)) * s + be.astype(np.float64)
    return s.astype(np.float32), t.astype(np.float32)


def _tile_w(Wb, ktiles, dtype):
    # Wb: [M_out, K_in] +-1, K_in padded to ktiles*128.
    # returns [M_out/128, 128, ktiles*128] with block (m, kp, k*128+mf)
    # = Wb[m*128+mf, k*128+kp]  (transposed lhsT tiles, contiguous per m)
    mo = Wb.shape[0] // 128
    return np.ascontiguousarray(
        Wb.reshape(mo, 128, ktiles, 128).transpose(0, 3, 2, 1).reshape(mo, 128, ktiles * 128)
    ).astype(dtype)


def _rtn11(x):
    # round fp32 to 11 explicit mantissa bits (round-to-nearest-even): the PE
    # fp32r path truncates to 11 explicit bits, so these pass through exactly
    m, e = np.frexp(x.astype(np.float64))
    m = np.round(m * 4096.0) / 4096.0
    return np.ldexp(m, e).astype(np.float32)


def kernel(**inputs):
    x = np.asarray(inputs["x"], dtype=np.float32)
    if "nc" not in _cache:
        _cache["nc"] = _build()
    nc = _cache["nc"]

    W1b = np.sign(np.asarray(inputs["W1"], np.float32))
    W2b = np.sign(np.asarray(inputs["W2"], np.float32))
    W3b = np.sign(np.asarray(inputs["W3"], np.float32))

    # main plane weights: rows 0:784 = W1b, pad rows 784:896 = W1b[:, 672:784]
    # (pad carries the exact residual of rows 672:784)
    W1p = np.concatenate([W1b, W1b[:, NCORR:IN]], axis=1)  # [H, 896]
    w1 = _tile_w(W1p, K1, np.float32)
    # corr weights, DR-paired: [m, ki, c*256 + j*128 + mf]
    #   = W1b[m*128+mf, c*256 + j*KCS[c] + ki] * 2^-12  (e5m2: exact, normal)
    w1cv = np.zeros((M1, 128, 768), np.float32)
    for c in range(3):
        kc = KCS[c]
        for j in range(2):
            # block [M1, 128(mf), kc(ki)] -> transpose to [M1, ki, mf]
            blk = W1b.reshape(M1, 128, IN)[:, :, c * 256 + j * kc : c * 256 + (j + 1) * kc]
            w1cv[:, :kc, c * 256 + j * 128 : c * 256 + j * 128 + 128] = (
                blk.transpose(0, 2, 1) * (1.0 / 4096.0)
            )
    w1cq = w1cv.astype(ml_dtypes.float8_e5m2)
    # w2 DoubleRow pairing: element (m, ki, kt*256 + j*128 + mf)
    #   = W2b[m*128+mf, (2*kt+j)*128 + ki]
    w2 = np.ascontiguousarray(
        W2b.reshape(M1, 128, KT2, 2, 128).transpose(0, 4, 2, 3, 1).reshape(M1, 128, H)
    ).astype(ml_dtypes.float8_e4m3)
    # w3 DR-paired: [kp, q*32 + j*16 + o] = W3b[o, (2q+j)*128 + kp], o<10
    w3v = np.zeros((128, KT2, 2, OPAD), np.float32)
    w3v[:, :, :, :OUT] = W3b.reshape(OUT, KT2, 2, 128).transpose(3, 1, 2, 0)
    w3 = np.ascontiguousarray(w3v.reshape(128, KT2 * 2 * OPAD)).astype(
        ml_dtypes.float8_e4m3
    )

    s1, t1 = _fold_bn(inputs["g1"], inputs["be1"], inputs["m1"], inputs["v1"], inputs["b1"])
    s2, t2 = _fold_bn(inputs["g2"], inputs["be2"], inputs["m2"], inputs["v2"], inputs["b2"])
    s3, t3 = _fold_bn(inputs["g3"], inputs["be3"], inputs["m3"], inputs["v3"], inputs["b3"])

    cst = np.zeros((128, 4 * M1 + 2), np.float32)
    cst[:, 0:M1] = s1.reshape(M1, 128).T
    cst[:, M1 : 2 * M1] = t1.reshape(M1, 128).T
    cst[:, 2 * M1 : 3 * M1] = s2.reshape(M1, 128).T
    cst[:, 3 * M1 : 4 * M1] = t2.reshape(M1, 128).T
    cst[:OUT, 4 * M1] = s3
    cst[:OUT, 4 * M1 + 1] = t3

    common = {"w1": w1, "w1c": w1cq, "w2": w2, "w3": w3, "cst": cst}

    xT = np.ascontiguousarray(x.T)          # [784, B]
    xb = _rtn11(xT)                         # main plane rows 0:784
    r = xT - xb                             # residual (rows 672:784 go to pad, exact)
    x0 = np.concatenate([xb, r[NCORR:IN, :]], axis=0)  # [896, B] fp32, 11-bit vals
    # corr planes: rows 0:672 residual * 2^12 in fp8e4m3, DR slot layout.
    # zero anything subnormal (|v| < 2^-6): the PE mangles subnormal fp8
    # stream operands; the lost correction is < 2^-18 absolute, negligible
    cqf = (r[:NCORR, :] * 4096.0).astype(ml_dtypes.float8_e4m3).astype(np.float32)
    cq = np.where(np.abs(cqf) < 2.0**-6, 0.0, cqf).astype(ml_dtypes.float8_e4m3)
    xc6 = np.zeros((128, 6, B), ml_dtypes.float8_e4m3)
    for c in range(3):
        kc = KCS[c]
        for j in range(2):
            xc6[0:kc, 2 * c + j, :] = cq[c * 256 + j * kc : c * 256 + (j + 1) * kc, :]

    in_maps = []
    for i in range(N_CORES):
        m = dict(common)
        sl = slice(i * BC, (i + 1) * BC)
        # chunk-contiguous repack: one DMA per chunk on device
        x0c = x0[:, sl].reshape(K1, 128, NCH, 512)
        m["x0"] = np.ascontiguousarray(
            x0c.transpose(1, 2, 0, 3).reshape(128, K1 * BC)
        )
        xcc = xc6[:, :, sl].reshape(128, 3, 2, NCH, 512)
        m["xc"] = np.ascontiguousarray(
            xcc.transpose(0, 3, 1, 2, 4).reshape(128, 6 * BC)
        )
        in_maps.append(m)

    try:
        res = run_bass_kernel_spmd(
            nc, in_maps, core_ids=list(range(N_CORES)), **_cache.get("run_kwargs", {})
        )
    except Exception:
        # transient NRT_EXEC_UNIT_UNRECOVERABLE has been observed on first
        # load after another NEFF; one retry has always recovered it
        res = run_bass_kernel_spmd(
            nc, in_maps, core_ids=list(range(N_CORES)), **_cache.get("run_kwargs", {})
        )
    _cache["last_results"] = res

    full = np.empty((B, OUT), np.float32)
    for i in range(N_CORES):
        full[i * BC : (i + 1) * BC, :] = res.results[i]["out"].T
    return full


# revision 40
# speedup vs baseline: 1.2024x; 1.2024x over previous
"""MnistDenseBNN Trainium2 kernel: 3-layer binarized MLP, data-parallel over 8 cores.

net: h1 = sign(BN1(x @ sign(W1).T + b1))
     h2 = sign(BN2(h1 @ sign(W2).T + b2))
     out = BN3(h2 @ sign(W3).T + b3)

Strategy (v3):
 - Shard batch (16384) 8 ways; replicate binarized weights + folded BN params.
 - PE streams 1 col/cycle regardless of dtype; fp8 DoubleRow packs K=256 per
   stream. Optimization = minimize 512-col streams per (m-tile, n-tile).
 - Layer 1 (10 streams/(m,n), was 14): main plane = rtn-to-11-mantissa-bits x
   (PE fp32r truncates to 11 explicit bits, so pre-rounded values pass exactly)
   over rows 0:784; pad rows 784:896 carry the residual of rows 672:784
   (exactly representable in 11 bits). Rows 0:672's residual goes through a
   fp8e4m3 DoubleRow correction (values r*2^9, weights +-2^-9) in 3 streams.
   Final-output rel err ~1.1e-2 vs the 2e-2 gate (verified in simulation).
 - Layer 2 (16 streams): +-1 x +-1 fp8e4m3 DoubleRow, K=256/stream. At floor.
 - Layer 3: fp8 DoubleRow too (16 streams/n-tile, was 32), fused into L2 loop.
 - BN+sign fused into one ScalarE activation (Sign(p*scale + bias)) from PSUM.
"""

import sys

sys.path.insert(0, "/opt/trn_rl_repo")

import numpy as np
import ml_dtypes
from ml_dtypes import bfloat16

import concourse.bass as bass
import concourse.bacc as bacc
import concourse.mybir as mybir
from concourse.tile import TileContext
from concourse.bass_utils import run_bass_kernel_spmd

N_CORES = 8
B, IN, H, OUT = 16384, 784, 4096, 10
BC = B // N_CORES          # 2048 batch per core
K1 = 7                     # 896 = 7*128 fp32r k-tiles for layer 1 main plane
KP = K1 * 128
NCORR = 672                # rows 0:672 corrected via fp8 DR; 672:784 exact via pad
KCS = (128, 128, 80)       # corr DR chunk partition sizes (pairs): 256+256+160 rows
M1 = H // 128              # 32 hidden tiles
KT2 = M1 // 2              # 16 DoubleRow k-steps in layer 2
OPAD = 16                  # layer-3 output slot padded 10 -> 16 (DR j-stride %16)
EPS = 1e-5

f32 = mybir.dt.float32
f32r = mybir.dt.float32r
bf16 = mybir.dt.bfloat16
fp8 = mybir.dt.float8e4
fp8e5 = mybir.dt.float8e5
AF = mybir.ActivationFunctionType
DR = mybir.MatmulPerfMode.DoubleRow

_cache = {}


def _ntiles(CH):
    # split a sub-chunk into n-tiles of 512 (tail 256)
    out, ns = [], 0
    while ns < CH:
        nw = min(512, CH - ns)
        out.append((ns, nw))
        ns += nw
    return out


NCH = BC // 512            # 4 chunks of 512 per core


def _build():
    nc = bacc.Bacc()
    # x main plane, chunk-contiguous: [p, g*K1*512 + k*512 + c] = xb[k*128+p, g*512+c]
    x0d = nc.dram_tensor("x0", [128, K1 * BC], f32r, kind="ExternalInput")
    # corr planes, chunk-contiguous DR pairs:
    # [p, g*3072 + c*1024 + j*512 + cc] = r[c*256 + j*KCS[c] + p, g*512+cc]*2^12
    xcd = nc.dram_tensor("xc", [128, 6 * BC], fp8, kind="ExternalInput")
    w1 = nc.dram_tensor("w1", [M1, 128, KP], f32r, kind="ExternalInput")
    # corr weights: [m, ki, c*256 + j*128 + mf] = sign(W1)[m*128+mf, k(c,j,ki)]*2^-12
    # e5m2 so +-2^-12 is a NORMAL fp8 value (e4m3 bottoms out at 2^-9; the PE
    # mangles subnormal fp8 operands)
    w1c = nc.dram_tensor("w1c", [M1, 128, 768], fp8e5, kind="ExternalInput")
    # w2: DoubleRow-paired fp8: [m, ki, kt*256 + j*128 + mf]
    #     = sign(W2)[m*128+mf, (2*kt+j)*128 + ki]
    w2 = nc.dram_tensor("w2", [M1, 128, H], fp8, kind="ExternalInput")
    # w3 DR-paired: [kp, q*32 + j*16 + o] = sign(W3)[o, (2q+j)*128 + kp] (o<10)
    w3 = nc.dram_tensor("w3", [128, KT2 * 2 * OPAD], fp8, kind="ExternalInput")
    # packed BN consts: cols [0:32]=s1 [32:64]=t1 [64:96]=s2 [96:128]=t2
    # col 128 rows 0:10 = s3, col 129 rows 0:10 = t3
    cst = nc.dram_tensor("cst", [128, 4 * M1 + 2], f32, kind="ExternalInput")
    out = nc.dram_tensor("out", [OUT, BC], f32, kind="ExternalOutput")

    with TileContext(nc) as tc:
        with (
            tc.tile_pool(name="consts", bufs=1) as cpool,
            tc.tile_pool(name="w1p", bufs=8) as w1pool,
            tc.tile_pool(name="w1cp", bufs=8) as w1cpool,
            tc.tile_pool(name="w2p", bufs=4) as w2pool,
            tc.tile_pool(name="xs", bufs=3) as xspool,
            tc.tile_pool(name="xc", bufs=3) as xcpool,
            tc.tile_pool(name="a1", bufs=2 * KT2) as a1pool,
            tc.tile_pool(name="a2", bufs=6) as a2pool,
            tc.tile_pool(name="ob", bufs=4) as opool,
            tc.tile_pool(name="ps1", bufs=3, space="PSUM") as ps1pool,
            tc.tile_pool(name="ps2", bufs=4, space="PSUM") as ps2pool,
            tc.tile_pool(name="ps3", bufs=1, space="PSUM") as ps3pool,
        ):
            # PE warmup on junk SBUF during the DMA head: HAM un-throttles
            # (1.2 -> 2.4 GHz) after ~3.4us of sustained matmul activity, so
            # the first real matmuls run warm instead of paying the cold ramp
            wup = cpool.tile([128, 512], bf16, tag="wup")
            nc.gpsimd.memset(wup, 0)
            psw = ps1pool.tile([128, 512], f32, tag="ps1", name="ps_warm")
            for i in range(60):
                nc.tensor.matmul(
                    psw, wup[:, :128], wup, start=(i == 0), stop=(i == 59)
                )

            # head: chunk-0 x and the first weight tiles go FIRST — the Sync
            # engine issues DMAs serially at ~0.6us each, so emission order
            # sets the critical path to the first matmul
            xst0 = xspool.tile([128, K1 * 512], f32r, tag="x0", name="xs_0")
            for h in range(K1):
                nc.sync.dma_start(
                    xst0[:, h * 512 : (h + 1) * 512], x0d[:, h * 512 : (h + 1) * 512]
                )
            w1t0 = w1pool.tile([128, KP], f32r, tag="w1")
            nc.sync.dma_start(w1t0, w1[0, :, :])
            # second HWDGE ring (Activation engine) issues in parallel
            w1ct0 = w1cpool.tile([128, 768], fp8e5, tag="w1c")
            nc.scalar.dma_start(w1ct0, w1c[0, :, :])
            xct0 = xcpool.tile([128, 6 * 512], fp8, tag="xc", name="xc_0")
            for h in range(3):
                nc.scalar.dma_start(
                    xct0[:, h * 1024 : (h + 1) * 1024],
                    xcd[:, h * 1024 : (h + 1) * 1024],
                )

            w3t = cpool.tile([128, KT2 * 2 * OPAD], fp8, tag="w3")
            nc.sync.dma_start(w3t, w3[:, :])
            cstt = cpool.tile([128, 4 * M1 + 2], f32, tag="cst")
            nc.sync.dma_start(cstt, cst[:, :])
            s1t = cstt[:, 0:M1]
            t1t = cstt[:, M1 : 2 * M1]
            s2t = cstt[:, 2 * M1 : 3 * M1]
            t2t = cstt[:, 3 * M1 : 4 * M1]
            s3t = cstt[:OUT, 4 * M1 : 4 * M1 + 1]
            t3t = cstt[:OUT, 4 * M1 + 1 : 4 * M1 + 2]
            # prime the scalar engine's clock on the const DMA queue so later
            # activations carry only their PE wait (1-wait walrus limit)
            prim = cpool.tile([128, 1], f32, tag="prim")
            nc.scalar.activation(prim, cstt[:, :1], AF.Copy)

            # 4 chunks of 512 batch-cols; each chunk's x arrives as ONE fp32r DMA
            # + ONE fp8 DMA (chunk-contiguous host layout)
            for g in range(NCH):
                cs = g * 512
                # ---- layer-1 input: fp32r main plane + fp8 DR corr planes ----
                if g == 0:
                    xst, xct = xst0, xct0
                else:
                    xst = xspool.tile(
                        [128, K1 * 512], f32r, tag="x0", name=f"xs_{g}"
                    )
                    for h in range(K1):
                        nc.sync.dma_start(
                            xst[:, h * 512 : (h + 1) * 512],
                            x0d[
                                :,
                                g * K1 * 512 + h * 512 : g * K1 * 512 + (h + 1) * 512,
                            ],
                        )
                    xct = xcpool.tile([128, 6 * 512], fp8, tag="xc", name=f"xc_{g}")
                    for h in range(3):
                        nc.scalar.dma_start(
                            xct[:, h * 1024 : (h + 1) * 1024],
                            xcd[:, g * 3072 + h * 1024 : g * 3072 + (h + 1) * 1024],
                        )

                # ---- layer 1: a1 = sign(s1*(x @ W1b^T) + t1), fp8 pair planes ----
                a1tiles = []
                for m in range(M1):
                    if g == 0 and m == 0:
                        w1t, w1ct = w1t0, w1ct0
                    else:
                        w1t = w1pool.tile([128, KP], f32r, tag="w1")
                        nc.sync.dma_start(w1t, w1[m, :, :])
                        w1ct = w1cpool.tile([128, 768], fp8e5, tag="w1c")
                        nc.scalar.dma_start(w1ct, w1c[m, :, :])
                    if m % 2 == 0:
                        a1tiles.append(
                            a1pool.tile(
                                [128, 2, 512], fp8, tag="a1",
                                name=f"a1dr_{g}_{m // 2}",
                            )
                        )
                    ps = ps1pool.tile([128, 512], f32, tag="ps1")
                    for k in range(K1):
                        nc.tensor.matmul(
                            ps,
                            w1t[:, k * 128 : (k + 1) * 128],
                            xst[:, k * 512 : (k + 1) * 512],
                            start=(k == 0),
                            stop=False,
                        )
                    for c in range(3):
                        kc = KCS[c]
                        nc.tensor.matmul(
                            ps,
                            w1ct[0:kc, c * 256 : (c + 1) * 256].rearrange(
                                "p (j m) -> p j m", j=2
                            ),
                            xct[0:kc, c * 1024 : (c + 1) * 1024].rearrange(
                                "p (j n) -> p j n", j=2
                            ),
                            start=False,
                            stop=(c == 2),
                            perf_mode=DR,
                        )
                    nc.scalar.activation(
                        a1tiles[m // 2][:, m % 2, :], ps, AF.Sign,
                        bias=t1t[:, m : m + 1], scale=s1t[:, m : m + 1],
                    )

                # ---- layer 2 (fp8 DoubleRow) + fused layer 3 (fp8 DR) ----
                psum3 = ps3pool.tile([OPAD, 512], f32, tag="ps3", name=f"ps3_{g}")
                a2cur = None
                for m in range(M1):
                    q = m // 2
                    # two half-tiles so same-tag DMA spacing stays 8-aligned
                    w2a = w2pool.tile([128, H // 2], fp8, tag="w2a")
                    nc.sync.dma_start(w2a, w2[m, :, : H // 2])
                    w2b = w2pool.tile([128, H // 2], fp8, tag="w2b")
                    nc.scalar.dma_start(w2b, w2[m, :, H // 2 :])
                    halves = (w2a, w2b)
                    ps = ps2pool.tile([128, 512], f32, tag="ps2")
                    for kt in range(KT2):
                        wt = halves[kt // (KT2 // 2)]
                        kk = kt % (KT2 // 2)
                        lhs3d = wt[:, kk * 256 : (kk + 1) * 256].rearrange(
                            "p (j m) -> p j m", j=2
                        )
                        rhs3d = a1tiles[kt][:, :, :]
                        nc.tensor.matmul(
                            ps, lhs3d, rhs3d,
                            start=(kt == 0), stop=(kt == KT2 - 1),
                            perf_mode=DR,
                        )
                    if m % 2 == 0:
                        a2cur = a2pool.tile(
                            [128, 2, 512], fp8, tag="a2", name=f"a2_{g}_{q}"
                        )
                    nc.scalar.activation(
                        a2cur[:, m % 2, :], ps, AF.Sign,
                        bias=t2t[:, m : m + 1], scale=s2t[:, m : m + 1],
                    )
                    if m % 2 == 1:
                        nc.tensor.matmul(
                            psum3,
                            w3t[:, q * 2 * OPAD : (q + 1) * 2 * OPAD].rearrange(
                                "p (j o) -> p j o", j=2
                            ),
                            a2cur[:, :, :],
                            start=(q == 0),
                            stop=(q == KT2 - 1),
                            perf_mode=DR,
                        )

                ob = opool.tile([OUT, 512], f32, tag="ob")
                nc.scalar.activation(
                    ob, psum3[:OUT, :], AF.Identity,
                    bias=t3t, scale=s3t,
                )
                nc.sync.dma_start(out[:, cs : cs + 512], ob)

    nc.finalize()
    return nc


def _fold_bn(g, be, mu, va, b):
    s = g.astype(np.float64) / np.sqrt(va.astype(np.float64) + EPS)
    t = (b.astype(np.float64) - mu.astype(np.float64)) * s + be.astype(np.float64)
    return s.astype(np.float32), t.astype(np.float32)


def _tile_w(Wb, ktiles, dtype):
    # Wb: [M_out, K_in] +-1, K_in padded to ktiles*128.
    # returns [M_out/128, 128, ktiles*128] with block (m, kp, k*128+mf)
    # = Wb[m*128+mf, k*128+kp]  (transposed lhsT tiles, contiguous per m)
    mo = Wb.shape[0] // 128
    return np.ascontiguousarray(
        Wb.reshape(mo, 128, ktiles, 128).transpose(0, 3, 2, 1).reshape(mo, 128, ktiles * 128)
    ).astype(dtype)


def _rtn11(x):
    # round fp32 to 11 explicit mantissa bits (round-to-nearest-even): the PE
    # fp32r path truncates to 11 explicit bits, so these pass through exactly
    m, e = np.frexp(x.astype(np.float64))
    m = np.round(m * 4096.0) / 4096.0
    return np.ldexp(m, e).astype(np.float32)


def kernel(**inputs):
    x = np.asarray(inputs["x"], dtype=np.float32)
    if "nc" not in _cache:
        _cache["nc"] = _build()
    nc = _cache["nc"]

    W1b = np.sign(np.asarray(inputs["W1"], np.float32))
    W2b = np.sign(np.asarray(inputs["W2"], np.float32))
    W3b = np.sign(np.asarray(inputs["W3"], np.float32))

    # main plane weights: rows 0:784 = W1b, pad rows 784:896 = W1b[:, 672:784]
    # (pad carries the exact residual of rows 672:784)
    W1p = np.concatenate([W1b, W1b[:, NCORR:IN]], axis=1)  # [H, 896]
    w1 = _tile_w(W1p, K1, np.float32)
    # corr weights, DR-paired: [m, ki, c*256 + j*128 + mf]
    #   = W1b[m*128+mf, c*256 + j*KCS[c] + ki] * 2^-12  (e5m2: exact, normal)
    w1cv = np.zeros((M1, 128, 768), np.float32)
    for c in range(3):
        kc = KCS[c]
        for j in range(2):
            # block [M1, 128(mf), kc(ki)] -> transpose to [M1, ki, mf]
            blk = W1b.reshape(M1, 128, IN)[:, :, c * 256 + j * kc : c * 256 + (j + 1) * kc]
            w1cv[:, :kc, c * 256 + j * 128 : c * 256 + j * 128 + 128] = (
                blk.transpose(0, 2, 1) * (1.0 / 4096.0)
            )
    w1cq = w1cv.astype(ml_dtypes.float8_e5m2)
    # w2 DoubleRow pairing: element (m, ki, kt*256 + j*128 + mf)
    #   = W2b[m*128+mf, (2*kt+j)*128 + ki]
    w2 = np.ascontiguousarray(
        W2b.reshape(M1, 128, KT2, 2, 128).transpose(0, 4, 2, 3, 1).reshape(M1, 128, H)
    ).astype(ml_dtypes.float8_e4m3)
    # w3 DR-paired: [kp, q*32 + j*16 + o] = W3b[o, (2q+j)*128 + kp], o<10
    w3v = np.zeros((128, KT2, 2, OPAD), np.float32)
    w3v[:, :, :, :OUT] = W3b.reshape(OUT, KT2, 2, 128).transpose(3, 1, 2, 0)
    w3 = np.ascontiguousarray(w3v.reshape(128, KT2 * 2 * OPAD)).astype(
        ml_dtypes.float8_e4m3
    )

    s1, t1 = _fold_bn(inputs["g1"], inputs["be1"], inputs["m1"], inputs["v1"], inputs["b1"])
    s2, t2 = _fold_bn(inputs["g2"], inputs["be2"], inputs["m2"], inputs["v2"], inputs["b2"])
    s3, t3 = _fold_bn(inputs["g3"], inputs["be3"], inputs["m3"], inputs["v3"], inputs["b3"])

    cst = np.zeros((128, 4 * M1 + 2), np.float32)
    cst[:, 0:M1] = s1.reshape(M1, 128).T
    cst[:, M1 : 2 * M1] = t1.reshape(M1, 128).T
    cst[:, 2 * M1 : 3 * M1] = s2.reshape(M1, 128).T
    cst[:, 3 * M1 : 4 * M1] = t2.reshape(M1, 128).T
    cst[:OUT, 4 * M1] = s3
    cst[:OUT, 4 * M1 + 1] = t3

    common = {"w1": w1, "w1c": w1cq, "w2": w2, "w3": w3, "cst": cst}

    xT = np.ascontiguousarray(x.T)          # [784, B]
    xb = _rtn11(xT)                         # main plane rows 0:784
    r = xT - xb                             # residual (rows 672:784 go to pad, exact)
    x0 = np.concatenate([xb, r[NCORR:IN, :]], axis=0)  # [896, B] fp32, 11-bit vals
    # corr planes: rows 0:672 residual * 2^12 in fp8e4m3, DR slot layout.
    # zero anything subnormal (|v| < 2^-6): the PE mangles subnormal fp8
    # stream operands; the lost correction is < 2^-18 absolute, negligible
    cqf = (r[:NCORR, :] * 4096.0).astype(ml_dtypes.float8_e4m3).astype(np.float32)
    cq = np.where(np.abs(cqf) < 2.0**-6, 0.0, cqf).astype(ml_dtypes.float8_e4m3)
    xc6 = np.zeros((128, 6, B), ml_dtypes.float8_e4m3)
    for c in range(3):
        kc = KCS[c]
        for j in range(2):
            xc6[0:kc, 2 * c + j, :] = cq[c * 256 + j * kc : c * 256 + (j + 1) * kc, :]

    in_maps = []
    for i in range(N_CORES):
        m = dict(common)
        sl = slice(i * BC, (i + 1) * BC)
        # chunk-contiguous repack: one DMA per chunk on device
        x0c = x0[:, sl].reshape(K1, 128, NCH, 512)
        m["x0"] = np.ascontiguousarray(
            x0c.transpose(1, 2, 0, 3).reshape(128, K1 * BC)
        )
        xcc = xc6[:, :, sl].reshape(128, 3, 2, NCH, 512)
        m["xc"] = np.ascontiguousarray(
            xcc.transpose(0, 3, 1, 2, 4).reshape(128, 6 * BC)
        )
        in_maps.append(m)

    try:
        res = run_bass_kernel_spmd(
            nc, in_maps, core_ids=list(range(N_CORES)), **_cache.get("run_kwargs", {})
        )
    except Exception:
        # transient NRT_EXEC_UNIT_UNRECOVERABLE has been observed on first
        # load after another NEFF; one retry has always recovered it
        res = run_bass_kernel_spmd(
            nc, in_maps, core_ids=list(range(N_CORES)), **_cache.get("run_kwargs", {})
        )
    _cache["last_results"] = res

    full = np.empty((B, OUT), np.float32)
    for i in range(N_CORES):
        full[i * BC : (i + 1) * BC, :] = res.results[i]["out"].T
    return full
